# revision 1
# baseline (speedup 1.0000x reference)
"""Causal multi-head attention block (B=2, S=2048, D=1024, H=16) on 8 TRN2 cores.

Sharding: core i handles batch b = i//4 and head group hg = i%4 (4 heads =
256 model dims). Each core computes its heads' attention and a partial
output projection; the host sums the 4 partials per batch and adds b_out.

Per-core device pipeline (bf16 matmuls, fp32 PSUM accumulation):
  1. QKV. Q^T,K^T land as [head_cols, tokens] (lhsT = W, rhs = x^T);
     V lands as [tokens, head_cols] (lhsT = x^T tiles, rhs = W_v) and is
     stored augmented with a ones column so the attention z-matmul also
     produces softmax row sums.
  2. Attention per head, flash-style in the S^T = K.Q^T orientation over
     the causal lower triangle only: S^T[k_tile, q_span] -> exp on ScalarE
     (scale=1/8; no max subtraction, logits are ~N(0,1)) -> P^T bf16 ->
     multiplicative 0/1 mask on the diagonal block -> z^T[d+1, q] +=
     V_aug^T @ P^T accumulated over k tiles in PSUM. Consecutive k tiles
     share one S region so each exp call covers up to 1024 columns.
  3. Normalize as soon as a q-quarter's last k tile lands: recip(rowsum),
     GPSIMD partition-broadcast, z * recip on VectorE -> bf16 zT. The V
     bias is folded into the output bias on the host (b_v @ w_out).
  4. Out-proj: y_partial[t, n] accumulated over the 256 local dims.

Program order is a 4-stage pipeline over 512-token quarters --
QKV(tg0), att(qg0), QKV(tg1), att(qg1), ... out-proj last -- so ScalarE
exp work overlaps PE QKV work and out-proj fills late PE gaps. Host
pre-packs all inputs into SBUF layouts (bf16) for contiguous DMA.
"""

import numpy as np
import ml_dtypes

import concourse.mybir as mybir
import concourse.tile as tile
from concourse import bacc
from concourse.bass_utils import run_bass_kernel_spmd

B = 2
S = 2048
DM = 1024
HD = 64
HLOC = 4                 # heads per core
CLOC = HLOC * HD         # local model dims (256)
MO = DM // 128           # 8 k-subtiles of the model dim
NKT = S // 128           # 16 key tiles


f32 = mybir.dt.float32
bf16 = mybir.dt.bfloat16
EXP = mybir.ActivationFunctionType.Exp

_CACHE = {}


def build(ps_z_bufs=2, pt_bufs=5, op_engines=("dve", "dve", "dve", "mix"),
          interleave_heads=False, dma_splits=(1, 1, 1, 4), sp_bufs=4,
          yp_bufs=4, sreg_w=1024):
    nc = bacc.Bacc("TRN2", target_bir_lowering=False, debug=False)

    xT_d = nc.dram_tensor("xT", [128, MO, S], bf16, kind="ExternalInput")
    wqk_d = nc.dram_tensor("wqk", [128, MO, 2 * CLOC], bf16, kind="ExternalInput")
    wv_d = nc.dram_tensor("wv", [128, MO, CLOC], bf16, kind="ExternalInput")
    wo_d = nc.dram_tensor("wo", [128, 2, DM], bf16, kind="ExternalInput")
    # consts packed as raw bf16 columns: tri[0:128], bq[128:132],
    # bk[132:136], bv[136:144] (f32 values bit-split across bf16 pairs)
    cst_d = nc.dram_tensor("cst", [128, 144], bf16, kind="ExternalInput")
    y_d = nc.dram_tensor("y", [S, DM], f32, kind="ExternalOutput")

    with tile.TileContext(nc) as tc:
        with (
            tc.tile_pool(name="consts", bufs=1) as consts,
            tc.tile_pool(name="acts", bufs=1) as apool,
            tc.tile_pool(name="pt", bufs=pt_bufs) as ppool,
            tc.tile_pool(name="norm", bufs=sp_bufs) as spool,
            tc.tile_pool(name="ycopy", bufs=yp_bufs) as ypool,
            # 8 PSUM banks: ps_s 2x[128,1024]=4 (QKV Q/K + attention S),
            # ps_z [128,512] z accumulators, ps_b rest (V / out-proj)
            tc.tile_pool(name="ps_s", bufs=2, space="PSUM") as ps_s,
            tc.tile_pool(name="ps_z", bufs=ps_z_bufs, space="PSUM") as ps_z,
            tc.tile_pool(name="ps_b", bufs=8 - 2 * (sreg_w // 512) - ps_z_bufs,
                         space="PSUM") as ps_b,
        ):
            csb = consts.tile([128, 144], bf16)
            wqk = consts.tile([128, MO, 2 * CLOC], bf16)
            wv = consts.tile([128, MO, CLOC], bf16)
            wo = consts.tile([128, 2, DM], bf16)
            xT = apool.tile([128, MO, S], bf16)

            # DMA order = consumption order. First chunks are fine-grained
            # so the first QKV matmul starts ~2.5us in; the rest are big
            # transfers to minimize per-DMA descriptor overhead.
            nc.sync.dma_start(wqk[:, 0:3, 0:256], wqk_d[:, 0:3, 0:256])
            nc.scalar.dma_start(xT[:, 0:3, 0:512], xT_d[:, 0:3, 0:512])
            nc.sync.dma_start(csb[:], cst_d[:])
            nc.sync.dma_start(wqk[:, 3:MO, 0:256], wqk_d[:, 3:MO, 0:256])
            nc.scalar.dma_start(xT[:, 3:MO, 0:512], xT_d[:, 3:MO, 0:512])
            nc.sync.dma_start(wqk[:, :, 256:512], wqk_d[:, :, 256:512])
            nc.scalar.dma_start(wv[:], wv_d[:])
            nc.sync.dma_start(xT[:, :, 512:1024], xT_d[:, :, 512:1024])
            nc.scalar.dma_start(xT[:, :, 1024:1536], xT_d[:, :, 1024:1536])
            nc.sync.dma_start(xT[:, :, 1536:2048], xT_d[:, :, 1536:2048])
            nc.scalar.dma_start(wo[:], wo_d[:])

            tri = csb[:, 0:128]
            bq_sb = csb[:, 128:132].bitcast(f32)
            bk_sb = csb[:, 132:136].bitcast(f32)

            QT = apool.tile([128, 2, S], bf16)
            KT = apool.tile([128, 2, S], bf16)
            # V augmented: [t-part, kt, h, 0:64] = v dims, col 64 = ones
            VA = apool.tile([128, NKT, HLOC, 72], bf16)
            nc.vector.memset(VA[:, :, :, 64:65], 1.0)
            zT = apool.tile([128, 2, S], bf16)

            def emit_qkv_qk(tg, pool=None, ptag="s"):
                pool = pool or ps_s
                tsl = slice(tg * 512, (tg + 1) * 512)
                for ct in range(2):
                    for j, (dst, b_sb) in enumerate(
                        ((QT, bq_sb), (KT, bk_sb))
                    ):
                        csl = slice(ct * 256 + j * 128, ct * 256 + (j + 1) * 128)
                        ps = pool.tile([128, 512 if ptag == "b" else sreg_w],
                                       f32, tag=ptag,
                                       name=f"qk_{tg}_{ct}_{j}")
                        for mo in range(MO):
                            nc.tensor.matmul(
                                ps[:, 0:512],
                                wqk[:, mo, csl],
                                xT[:, mo, tsl],
                                start=(mo == 0),
                                stop=(mo == MO - 1),
                            )
                        nc.vector.tensor_scalar_add(
                            dst[:, ct, tsl], ps[:, 0:512], b_sb[:, ct : ct + 1]
                        )
            def emit_qkv_v(tg):
                for ti in range(4):
                    tt = tg * 4 + ti
                    ps = ps_b.tile([128, 512], f32, tag="b")
                    for mo in range(MO):
                        nc.tensor.matmul(
                            ps[:, 0:CLOC],
                            xT[:, mo, tt * 128 : (tt + 1) * 128],
                            wv[:, mo, :],
                            start=(mo == 0),
                            stop=(mo == MO - 1),
                        )
                    if tg == 0:
                        nc.scalar.copy(
                            VA[:, tt, :, 0:64],
                            ps[:, 0:CLOC].rearrange("p (h d) -> p h d", d=64),
                        )
                    else:
                        nc.vector.tensor_copy(
                            VA[:, tt, :, 0:64],
                            ps[:, 0:CLOC].rearrange("p (h d) -> p h d", d=64),
                        )

            def emit_attention(qg):
                g0 = qg * 512
                last_kt = 4 * qg + 3

                # pack consecutive k tiles into shared S regions so one
                # exp call covers up to 1024 columns
                groups, cur, cum = [], [], 0
                for kt in range(last_kt + 1):
                    w = g0 + 512 - max(kt * 128, g0)
                    if cum + w > sreg_w:
                        groups.append(cur)
                        cur, cum = [], 0
                    cur.append((kt, cum, w))
                    cum += w
                groups.append(cur)

                head_lists = ([0, 1, 3, 2] if not interleave_heads
                              else [[0, 1], [2, 3]])
                for hl in head_lists:
                    hs = [hl] if isinstance(hl, int) else hl
                    zp_map = {}
                    for h in hs:
                        zp_map[h] = ps_z.tile([128, 512], f32, tag="z",
                                              name=f"zps_{h}_{qg}")
                    for grp in groups:
                        for h in hs:
                            emit_head_grp(h, grp, zp_map[h], qg, g0, last_kt)
                    for h in hs:
                        emit_norm(h, zp_map[h], qg, g0)

            def emit_head_grp(h, grp, zp, qg, g0, last_kt):
                    hp = (h % 2) * 64
                    ct = h // 2
                    if True:
                        sreg = ps_s.tile([128, sreg_w], f32, tag="s",
                                         name=f"s_{h}_{qg}_{grp[0][0]}")
                        cum = grp[-1][1] + grp[-1][2]
                        for kt, off, w in grp:
                            q0 = g0 + 512 - w
                            c0 = off
                            while c0 < off + w:
                                cw = min(off + w - c0, 512 - c0 % 512)
                                nc.tensor.matmul(
                                    sreg[:, c0 : c0 + cw],
                                    KT[hp : hp + 64, ct,
                                       kt * 128 : (kt + 1) * 128],
                                    QT[hp : hp + 64, ct,
                                       q0 + c0 - off : q0 + c0 - off + cw],
                                )
                                c0 += cw
                        pT = ppool.tile([128, sreg_w], bf16, tag="pT")
                        nc.scalar.activation(
                            pT[:, :cum], sreg[:, :cum], EXP, scale=0.125
                        )
                        for kt, off, w in grp:
                            if kt * 128 >= g0:  # diagonal block leads span
                                nc.vector.tensor_mul(
                                    pT[:, off : off + 128],
                                    pT[:, off : off + 128],
                                    tri[:],
                                )
                            q0 = g0 + 512 - w
                            nc.tensor.matmul(
                                zp[0:65, q0 - g0 : 512],
                                VA[:, kt, h, 0:65],
                                pT[:, off : off + w],
                                start=(kt == 0),
                                stop=(kt == last_kt),
                            )

            def emit_norm(h, zp, qg, g0):
                    hp = (h % 2) * 64
                    ct = h // 2
                    rec32 = spool.tile([1, 512], f32, tag="rec32",
                                       name=f"rec_{h}_{qg}")
                    nc.vector.reciprocal(rec32[:], zp[64:65, 0:512])
                    bcast = spool.tile([64, 512], f32, tag="bcast",
                                       name=f"bc_{h}_{qg}")
                    nc.gpsimd.partition_broadcast(bcast[:], rec32[:])
                    # b_v is folded into b_out on the host:
                    # y += (1 (x) b_v) @ w_out is a constant row vector
                    with nc.allow_low_precision(reason="attn out to bf16"):
                        if hp == 0:
                            nc.vector.tensor_mul(
                                zT[0:64, ct, g0 : g0 + 512],
                                zp[0:64, 0:512], bcast[:],
                            )
                        else:
                            zbf = spool.tile([64, 512], bf16, tag="zbf",
                                             name=f"zb_{h}_{qg}")
                            nc.vector.tensor_mul(
                                zbf[:], zp[0:64, 0:512], bcast[:]
                            )
                            nc.sync.dma_start(
                                zT[hp : hp + 64, ct, g0 : g0 + 512], zbf[:]
                            )

            def emit_outproj(qg, copy_eng, dma_split=1):
                for nh in range(2):
                    ysb = ypool.tile([128, 4, 512], f32, tag="y",
                                     name=f"ysb_{qg}_{nh}")
                    nper = 4 // dma_split
                    for ti in range(4):
                        tt = qg * 4 + ti
                        ps = ps_b.tile([128, 512], f32, tag="b")
                        for co in range(2):
                            nc.tensor.matmul(
                                ps[:],
                                zT[:, co, tt * 128 : (tt + 1) * 128],
                                wo[:, co, nh * 512 : (nh + 1) * 512],
                                start=(co == 0),
                                stop=(co == 1),
                            )
                        eng = copy_eng if copy_eng != "mix" else (
                            "act" if (tt + nh) % 2 == 0 else "dve"
                        )
                        if eng == "act":
                            nc.scalar.copy(ysb[:, ti, :], ps[:])
                        else:
                            nc.vector.tensor_copy(ysb[:, ti, :], ps[:])
                        if ti % nper == nper - 1:
                            t0 = tt - nper + 1
                            deng = nc.sync if (ti // nper + nh) % 2 == 0 else nc.scalar
                            deng.dma_start(
                                y_d[t0 * 128 : (tt + 1) * 128,
                                    nh * 512 : (nh + 1) * 512].rearrange(
                                    "(ti p) n -> p ti n", p=128
                                ),
                                ysb[:, ti - nper + 1 : ti + 1, :],
                            )

            # 4-stage software pipeline: attention on quarter qg overlaps
            # the QKV projection of quarter qg+1 on PE
            emit_qkv_qk(0)
            emit_qkv_v(0)
            emit_attention(0)
            emit_qkv_qk(1)
            emit_qkv_v(1)
            emit_attention(1)
            emit_qkv_qk(2)
            emit_qkv_v(2)
            emit_attention(2)
            emit_qkv_qk(3)
            emit_qkv_v(3)
            emit_attention(3)
            for qg in range(4):
                emit_outproj(qg, op_engines[qg], dma_split=dma_splits[qg])

    nc.compile()
    return nc


def _pack_w(w):
    # [DM, C] -> [128, MO, C]: partition p holds rows {mo*128 + p}
    return np.ascontiguousarray(
        w.reshape(MO, 128, w.shape[1]).transpose(1, 0, 2)
    ).astype(ml_dtypes.bfloat16)


def make_in_maps(x, w_qkv, b_qkv, w_out):
    # multiplicative post-exp mask: 1 where k <= q (upper incl diag), else 0
    tri = np.tri(128, 128, 0, dtype=np.float32).T.astype(ml_dtypes.bfloat16)
    in_maps = []
    for core in range(8):
        b = core // 4
        hg = core % 4
        c0 = hg * CLOC
        csl = slice(c0, c0 + CLOC)

        # packed consts: [128, 144] bf16-typed raw columns
        cst = np.zeros((128, 144), np.uint16)
        cst[:, 0:128] = tri.view(np.uint16)
        bq = np.ascontiguousarray(
            b_qkv[csl].astype(np.float32).reshape(2, 128).T
        )
        bk = np.ascontiguousarray(
            b_qkv[DM + c0 : DM + c0 + CLOC].astype(np.float32).reshape(2, 128).T
        )
        bv = np.ascontiguousarray(
            b_qkv[2 * DM + c0 : 2 * DM + c0 + CLOC]
            .astype(np.float32).reshape(HLOC, 64).T
        )
        cst[:, 128:132] = bq.view(np.uint16).reshape(128, 4)
        cst[:, 132:136] = bk.view(np.uint16).reshape(128, 4)
        cst[0:64, 136:144] = bv.view(np.uint16).reshape(64, 8)

        wq_p = _pack_w(w_qkv[:, csl])
        wk_p = _pack_w(w_qkv[:, DM + c0 : DM + c0 + CLOC])
        wqk = np.concatenate(
            [wq_p[:, :, 0:128], wk_p[:, :, 0:128],
             wq_p[:, :, 128:256], wk_p[:, :, 128:256]],
            axis=2,
        )
        in_maps.append(
            {
                "xT": _pack_w(np.ascontiguousarray(x[b].T)),
                "wqk": np.ascontiguousarray(wqk),
                "wv": _pack_w(w_qkv[:, 2 * DM + c0 : 2 * DM + c0 + CLOC]),
                # wo: [CLOC, DM] -> [128, 2, DM]
                "wo": np.ascontiguousarray(
                    w_out[csl, :].reshape(2, 128, DM).transpose(1, 0, 2)
                ).astype(ml_dtypes.bfloat16),
                "cst": cst.view(ml_dtypes.bfloat16),
            }
        )
    return in_maps


def gather(results, b_qkv, w_out, b_out):
    # device skips the V bias; z_norm + b_v projects to a constant row:
    # y += b_v @ w_out, folded into the output bias here
    b_eff = (
        b_out.astype(np.float32)
        + b_qkv[2 * DM :].astype(np.float32) @ w_out.astype(np.float32)
    )
    out = np.empty((B, S, DM), np.float32)
    for b in range(B):
        acc = results[4 * b]["y"].astype(np.float32)
        for j in range(1, 4):
            acc = acc + results[4 * b + j]["y"]
        out[b] = acc + b_eff[None, :]
    return out


def kernel(x, w_qkv, b_qkv, w_out, b_out):
    x = np.asarray(x)
    w_qkv = np.asarray(w_qkv)
    b_qkv = np.asarray(b_qkv)
    w_out = np.asarray(w_out)
    b_out = np.asarray(b_out)

    if "nc" not in _CACHE:
        _CACHE["nc"] = build()
    nc = _CACHE["nc"]

    in_maps = make_in_maps(x, w_qkv, b_qkv, w_out)
    res = run_bass_kernel_spmd(nc, in_maps, core_ids=list(range(8)))
    return gather(res.results, b_qkv, w_out, b_out)



# revision 39
# speedup vs baseline: 1.1103x; 1.1103x over previous
"""Causal multi-head attention block (B=2, S=2048, D=1024, H=16) on 8 TRN2 cores.

Sharding: core i handles batch b = i//4 and head group hg = i%4 (4 heads =
256 model dims). Each core computes its heads' attention and a partial
output projection; the host sums the 4 partials per batch and adds b_out.

Per-core device pipeline (bf16 matmuls, fp32 PSUM accumulation):
  1. QKV. Q^T,K^T land as [head_cols, tokens] (lhsT = W, rhs = x^T);
     V lands as [tokens, head_cols] (lhsT = x^T tiles, rhs = W_v), stored
     augmented with a ones column so the z-matmul also emits row sums.
  2. Attention per head over the causal lower triangle only:
     S^T[k_tile, q_span] -> exp on ScalarE (scale=1/8, no max subtraction;
     logits are ~N(0,1)) -> P^T bf16 -> multiplicative 0/1 mask on diagonal
     blocks (GPSIMD) -> z[q, d+1] += P^T.T @ V_aug accumulated over k tiles
     in PSUM. The z orientation pays only 65 PE columns per (q,k) tile pair
     (vs 128 for z^T), at the cost of a transpose before the out-proj.
  3. Normalize per q-tile as soon as its last k tile lands: recip(rowsum
     column), z * recip on VectorE -> bf16 zn[q, c]; PE transpose (identity
     matmul) -> zT[c, q] for the out-proj lhsT.
  4. Out-proj: y_partial[t, n] over the 256 local dims, staged to SBUF as
     bf16 and DMA'd out (host sums partials in f32). The V bias is folded
     into the output bias on the host (b_v @ w_out).

Scheduling: a filler queue of QKV chains (next quarter) and out-proj units
(previous quarter) is pumped between attention rounds to keep PE busy while
ScalarE works through the exp chain, which is the attention-phase critical
path. Host pre-packs all inputs into SBUF layouts (bf16) for contiguous DMA.
"""

import numpy as np
import ml_dtypes

import concourse.mybir as mybir
import concourse.tile as tile
from concourse import bacc
from concourse.bass_utils import run_bass_kernel_spmd

B = 2
S = 2048
DM = 1024
HD = 64
HLOC = 4                 # heads per core
CLOC = HLOC * HD         # local model dims (256)
MO = DM // 128           # 8 k-subtiles of the model dim
NKT = S // 128           # 16 key tiles

f32 = mybir.dt.float32
bf16 = mybir.dt.bfloat16
EXP = mybir.ActivationFunctionType.Exp

_CACHE = {}

PE_NS = 1.0 / 2.4        # ns per PE column-cycle at full clock
ACT_NS = 1.0 / 1.2

# dev-only instruction labels for trace attribution (KLBL=1)
LABELS = {}
_ctx = ["?"]


def qg_groups(qg, sreg_w):
    """Pack k tiles 0..4qg+3 into shared S regions of width <= sreg_w."""
    groups, cur, cum = [], [], 0
    for kt in range(4 * qg + 4):
        w = 512 if kt < 4 * qg else 512 - (kt - 4 * qg) * 128
        if cum + w > sreg_w:
            groups.append(cur)
            cur, cum = [], 0
        cur.append((kt, cum, w))
        cum += w
    groups.append(cur)
    return groups


def build(sreg_w=1024, pt_bufs=8, ps_z_bufs=2, ps_b_bufs=2, zn_bufs=2,
          sp_bufs=6, yp_bufs=4, exp_call_ns=185.0, pump_bias=0.0,
          tr_pump_ns=1500.0, zlag=1, mask_eng="dve", act_scale=0.95,
          credit=False):
    nc = bacc.Bacc("TRN2", target_bir_lowering=False, debug=False)

    import os
    if os.environ.get("KLBL"):
        _orig_mm = nc.tensor.matmul

        def _mm(*a, **k):
            inst = _orig_mm(*a, **k)
            LABELS[inst.ins.name] = _ctx[0]
            return inst

        nc.tensor.matmul = _mm

    xT_d = nc.dram_tensor("xT", [128, MO, S], bf16, kind="ExternalInput")
    # wqk chunk-major: chunk c in (Q ct0, K ct0, Q ct1, K ct1), then mo
    wqk_d = nc.dram_tensor("wqk", [128, 4, MO, 128], bf16, kind="ExternalInput")
    wv_d = nc.dram_tensor("wv", [128, MO, CLOC], bf16, kind="ExternalInput")
    wo_d = nc.dram_tensor("wo", [128, 2, DM], bf16, kind="ExternalInput")
    # consts: tri[0:128], identity[128:256], bq[256:260], bk[260:264]
    # (bias f32 values bit-split across bf16 pairs)
    cst_d = nc.dram_tensor("cst", [128, 264], bf16, kind="ExternalInput")
    y_d = nc.dram_tensor("y", [S, DM], bf16, kind="ExternalOutput")

    with tile.TileContext(nc) as tc:
        with (
            tc.tile_pool(name="consts", bufs=1) as consts,
            tc.tile_pool(name="acts", bufs=1) as apool,
            tc.tile_pool(name="pt", bufs=pt_bufs) as ppool,
            tc.tile_pool(name="norm", bufs=sp_bufs) as spool,
            tc.tile_pool(name="zn", bufs=zn_bufs) as znpool,
            tc.tile_pool(name="ycopy", bufs=yp_bufs) as ypool,
            # 8 PSUM banks: ps_s 2x[128,sreg_w] (S regions), ps_z z
            # accumulators, ps_b the rest (QKV / out-proj / transposes)
            tc.tile_pool(name="ps_s", bufs=2, space="PSUM") as ps_s,
            tc.tile_pool(name="ps_z", bufs=ps_z_bufs, space="PSUM") as ps_z,
            tc.tile_pool(name="ps_b", bufs=ps_b_bufs, space="PSUM") as ps_b,
        ):
            csb = consts.tile([128, 264], bf16)
            wqk = consts.tile([128, 4, MO, 128], bf16)
            wv = consts.tile([128, MO, CLOC], bf16)
            wo = consts.tile([128, 2, DM], bf16)
            xT = apool.tile([128, MO, S], bf16)

            # DMA order = consumption order; first chunks sized so the first
            # QKV chain starts ~3us in. HWDGE + the transfer bus serialize,
            # so volume order matters more than queue choice.
            nc.sync.dma_start(wqk[:, 0, :, :], wqk_d[:, 0, :, :])
            nc.scalar.dma_start(xT[:, 0:2, 0:512], xT_d[:, 0:2, 0:512])
            nc.sync.dma_start(xT[:, 2:5, 0:512], xT_d[:, 2:5, 0:512])
            nc.scalar.dma_start(wqk[:, 1, :, :], wqk_d[:, 1, :, :])
            nc.sync.dma_start(xT[:, 5:8, 0:512], xT_d[:, 5:8, 0:512])
            nc.scalar.dma_start(wqk[:, 2:4, :, :], wqk_d[:, 2:4, :, :])
            nc.sync.dma_start(wv[:], wv_d[:])
            nc.scalar.dma_start(csb[:], cst_d[:])
            nc.sync.dma_start(xT[:, :, 512:1024], xT_d[:, :, 512:1024])
            nc.scalar.dma_start(wo[:], wo_d[:])
            nc.sync.dma_start(xT[:, :, 1024:1536], xT_d[:, :, 1024:1536])
            nc.scalar.dma_start(xT[:, :, 1536:2048], xT_d[:, :, 1536:2048])

            tri = csb[:, 0:128]
            ident = csb[:, 128:256]
            bq_sb = csb[:, 256:260].bitcast(f32)
            bk_sb = csb[:, 260:264].bitcast(f32)

            QT = apool.tile([128, 2, S], bf16)
            KT = apool.tile([128, 2, S], bf16)
            # V augmented: [t-part, kt, h, 0:64] = v dims, col 64 = ones
            VA = apool.tile([128, NKT, HLOC, 72], bf16)
            nc.vector.memset(VA[:, :, :, 64:65], 1.0)
            zT = apool.tile([128, 2, S], bf16)

            # ---------------- emission helpers ----------------

            def emit_qkv_qk_chain(tg, ct, j):
                _ctx[0] = f"qk{tg}"
                tsl = slice(tg * 512, (tg + 1) * 512)
                c = ct * 2 + j
                dst, b_sb = ((QT, bq_sb), (KT, bk_sb))[j]
                ps = ps_b.tile([128, 512], f32, tag="b",
                               name=f"qk_{tg}_{ct}_{j}")
                for mo in range(MO):
                    nc.tensor.matmul(
                        ps[:],
                        wqk[:, c, mo, :],
                        xT[:, mo, tsl],
                        start=(mo == 0),
                        stop=(mo == MO - 1),
                    )
                nc.vector.tensor_scalar_add(
                    dst[:, ct, tsl], ps[:], b_sb[:, ct : ct + 1]
                )

            def emit_qkv_v_chain(tg, ti):
                _ctx[0] = f"v{tg}"
                tt = tg * 4 + ti
                ps = ps_b.tile([128, 512], f32, tag="b", name=f"v_{tt}")
                for mo in range(MO):
                    nc.tensor.matmul(
                        ps[:, 0:CLOC],
                        xT[:, mo, tt * 128 : (tt + 1) * 128],
                        wv[:, mo, :],
                        start=(mo == 0),
                        stop=(mo == MO - 1),
                    )
                src = ps[:, 0:CLOC].rearrange("p (h d) -> p h d", d=64)
                if tg == 0:
                    nc.scalar.copy(VA[:, tt, :, 0:64], src)
                else:
                    nc.vector.tensor_copy(VA[:, tt, :, 0:64], src)

            def emit_s_grp(h, qg, grp, sreg):
                _ctx[0] = f"S{qg}g{grp[0][0]}h{h}"
                g0 = qg * 512
                hp = (h % 2) * 64
                ct = h // 2
                for kt, off, w in grp:
                    q0 = max(kt * 128, g0)
                    c0 = off
                    while c0 < off + w:
                        cw = min(off + w - c0, 512 - c0 % 512)
                        nc.tensor.matmul(
                            sreg[:, c0 : c0 + cw],
                            KT[hp : hp + 64, ct, kt * 128 : (kt + 1) * 128],
                            QT[hp : hp + 64, ct,
                               q0 + c0 - off : q0 + c0 - off + cw],
                        )
                        c0 += cw

            def emit_exp_mask(h, qg, grp, sreg, pT):
                g0 = qg * 512
                cum = grp[-1][1] + grp[-1][2]
                nc.scalar.activation(pT[:, :cum], sreg[:, :cum], EXP,
                                     scale=0.125)
                for kt, off, w in grp:
                    if kt * 128 >= g0:  # diagonal block leads its span
                        eng = nc.vector if mask_eng == "dve" else (
                            nc.gpsimd if mask_eng == "pool"
                            else (nc.vector if h % 2 == 0 else nc.gpsimd))
                        eng.tensor_mul(
                            pT[:, off : off + 128],
                            pT[:, off : off + 128],
                            tri[:],
                        )

            def emit_z_grp(h, qg, grp, pT, zp, norm_list):
                _ctx[0] = f"z{qg}g{grp[0][0]}h{h}"
                g0 = qg * 512
                # one accumulation group per zp bank: start_tensor_calc
                # lazily zeroes the whole 2KB zero region, so only the very
                # first matmul into the tile may carry start=True
                for kt, off, w in grp:
                    ti0 = (max(kt * 128, g0) - g0) // 128
                    for qt in range(ti0, 4):
                        last = kt == 4 * qg + qt
                        nc.tensor.matmul(
                            zp[:, qt, 0:65],
                            pT[:, off + (qt - ti0) * 128 :
                               off + (qt - ti0) * 128 + 128],
                            VA[:, kt, h, 0:65],
                            start=(kt == 0 and qt == 0),
                            stop=(kt == 4 * qg + 3 and qt == 3),
                            skip_group_check=True,
                        )
                        if last:
                            norm_list.append((h, qt))

            def emit_norm(h, qg, qt, zp, zn):
                hp = (h % 2) * 64
                ct = h // 2
                rec = spool.tile([128, 1], f32, tag="rec",
                                 name=f"rec_{qg}_{h}_{qt}")
                nc.vector.reciprocal(rec[:], zp[:, qt, 64:65])
                with nc.allow_low_precision(reason="attn out to bf16"):
                    nc.vector.tensor_scalar_mul(
                        zn[:, qt, ct, hp : hp + 64], zp[:, qt, 0:64], rec[:]
                    )

            def emit_transpose(qg, ct, qt, zn):
                _ctx[0] = f"tr{qg}c{ct}"
                g0 = qg * 512
                tp = ps_b.tile([128, 128], bf16, tag="b",
                               name=f"tp_{qg}_{ct}_{qt}")
                nc.tensor.transpose(tp[:], zn[:, qt, ct, :], ident)
                nc.vector.tensor_copy(
                    zT[:, ct, g0 + qt * 128 : g0 + (qt + 1) * 128], tp[:]
                )

            ysb_map = {}

            def emit_outproj_unit(qg, nh, ti, last=False):
                _ctx[0] = f"op{qg}"
                if ti == 0:
                    ysb_map[(qg, nh)] = ypool.tile(
                        [128, 4, 512], bf16, tag="y", name=f"ysb_{qg}_{nh}"
                    )
                ysb = ysb_map[(qg, nh)]
                tt = qg * 4 + ti
                ps = ps_b.tile([128, 512], f32, tag="b", name=f"op_{tt}_{nh}")
                for co in range(2):
                    nc.tensor.matmul(
                        ps[:],
                        zT[:, co, tt * 128 : (tt + 1) * 128],
                        wo[:, co, nh * 512 : (nh + 1) * 512],
                        start=(co == 0),
                        stop=(co == 1),
                    )
                with nc.allow_low_precision(reason="partial y to bf16"):
                    if qg == 3 and ti >= 2:
                        # after the last exp Act is free; DVE still norms
                        nc.scalar.copy(ysb[:, ti, :], ps[:])
                    else:
                        nc.vector.tensor_copy(ysb[:, ti, :], ps[:])
                if qg == 3:
                    # per-tile stores at the end so the drain tail is short
                    nc.sync.dma_start(
                        y_d[tt * 128 : (tt + 1) * 128,
                            nh * 512 : (nh + 1) * 512].rearrange(
                            "(a p) n -> p a n", p=128
                        ),
                        ysb[:, ti : ti + 1, :],
                    )
                elif ti % 2 == 1:
                    nc.sync.dma_start(
                        y_d[(tt - 1) * 128 : (tt + 1) * 128,
                            nh * 512 : (nh + 1) * 512].rearrange(
                            "(a p) n -> p a n", p=128
                        ),
                        ysb[:, ti - 1 : ti + 1, :],
                    )

            # ---------------- filler pump ----------------

            # Filler queue: each entry (pe_ns, tg, kind, fn). QKV chains are
            # consumed FIFO by the pump; forced-flush guards emit a quarter's
            # Q chains right before its attention phase and its K/V chains
            # right before the phase's diagonal groups, so Act never idles
            # waiting on a bulk qkv flush. Out-proj backlog fills the rest.
            fill_qkv = []
            fill_out = []
            state = {"deficit": pump_bias}

            def pump(extra_ns):
                state["deficit"] += extra_ns
                while state["deficit"] > 0 and (fill_qkv or fill_out):
                    q = fill_qkv if fill_qkv else fill_out
                    ent = q.pop(0)
                    ent[-1]()
                    state["deficit"] -= ent[0]

            def force_qkv(tg, kinds):
                keep = []
                for ent in fill_qkv:
                    if ent[1] == tg and ent[2] in kinds:
                        ent[-1]()
                        if credit:
                            state["deficit"] -= ent[0]
                    else:
                        keep.append(ent)
                fill_qkv[:] = keep

            def enqueue_qkv(tg):
                for ct, j in ((0, 0), (0, 1), (1, 0), (1, 1)):
                    fill_qkv.append(
                        (8 * 512 * PE_NS, tg, "QK"[j],
                         lambda tg=tg, ct=ct, j=j:
                         emit_qkv_qk_chain(tg, ct, j))
                    )
                for ti in range(4):
                    fill_qkv.append(
                        (8 * 256 * PE_NS, tg, "V",
                         lambda tg=tg, ti=ti:
                         emit_qkv_v_chain(tg, ti))
                    )

            def enqueue_outproj(qg):
                for nh in range(2):
                    for ti in range(4):
                        fill_out.append(
                            (2 * 512 * PE_NS, qg, "O",
                             lambda qg=qg, nh=nh, ti=ti:
                             emit_outproj_unit(qg, nh, ti))
                        )

            # ---------------- attention ----------------

            def emit_attention(qg):
                force_qkv(qg, ("Q",))
                groups = qg_groups(qg, sreg_w)
                g0 = qg * 512
                zn = znpool.tile([128, 4, 2, 128], bf16, tag="zn",
                                 name=f"zn_{qg}")
                for wi, heads in enumerate(((0, 1), (2, 3))):
                    zp = {
                        h: ps_z.tile([128, 4, 65], f32, tag="z",
                                     name=f"z_{qg}_{h}")
                        for h in heads
                    }
                    normed = {qt: 0 for qt in range(4)}
                    pending = []   # transposes (and qg3 out-proj) lagging
                                   # one round behind their norms

                    def note_norms(znorms):
                        for h, qt in znorms:
                            emit_norm(h, qg, qt, zp[h], zn)
                            normed[qt] += 1
                            if normed[qt] == 2:
                                pending.append(
                                    (0.0, lambda qt=qt:
                                     emit_transpose(qg, wi, qt, zn)))
                                if qg == 3 and wi == 1:
                                    for nh in range(2):
                                        pending.append(
                                            (2 * 512 * PE_NS,
                                             lambda nh=nh, qt=qt:
                                             emit_outproj_unit(3, nh, qt)))

                    def emit_z_round(pgrp, ppts):
                        znorms = []
                        for h in heads:
                            emit_z_grp(h, qg, pgrp, ppts[h], zp[h], znorms)
                        zcols = sum(
                            65 * (4 - (max(kt * 128, g0) - g0) // 128)
                            for kt, _, _ in pgrp
                        ) * 2
                        note_norms(znorms)
                        return zcols

                    backlog = []
                    for gi, grp in enumerate(groups):
                        if grp[-1][0] >= 4 * qg:
                            force_qkv(qg, ("K", "V"))
                        cum = grp[-1][1] + grp[-1][2]
                        cur = {}
                        for h in heads:
                            sreg = ps_s.tile([128, sreg_w], f32, tag="s",
                                             name=f"s_{qg}_{h}_{grp[0][0]}")
                            pT = ppool.tile([128, sreg_w], bf16, tag="pT",
                                            name=f"p_{qg}_{h}_{grp[0][0]}")
                            emit_s_grp(h, qg, grp, sreg)
                            emit_exp_mask(h, qg, grp, sreg, pT)
                            cur[h] = pT
                        ready, pending[:] = pending[:], []
                        backlog.append((grp, cur))
                        zcols = 0
                        if len(backlog) > zlag:
                            zcols = emit_z_round(*backlog.pop(0))
                        for pe_ns_r, fn in ready:
                            fn()
                            if credit:
                                state["deficit"] -= pe_ns_r
                        act_ns = 2 * (cum * ACT_NS + exp_call_ns) * act_scale
                        pe_ns = (2 * cum + zcols) * PE_NS
                        pump(act_ns - pe_ns)
                    while backlog:
                        emit_z_round(*backlog.pop(0))
                        pump(tr_pump_ns if len(backlog) == 0 else 500.0)
                        ready, pending[:] = pending[:], []
                        for pe_ns_r, fn in sorted(ready, key=lambda e: e[0]):
                            fn()
                            if credit:
                                state["deficit"] -= pe_ns_r
                    for pe_ns_r, fn in sorted(pending, key=lambda e: e[0]):
                        fn()
                    pending[:] = []

            # ---------------- program ----------------

            for ct, j in ((0, 0), (0, 1), (1, 0), (1, 1)):
                emit_qkv_qk_chain(0, ct, j)
            for ti in range(4):
                emit_qkv_v_chain(0, ti)

            for qg in range(4):
                if qg < 3:
                    enqueue_qkv(qg + 1)
                emit_attention(qg)
                if qg < 3:
                    enqueue_outproj(qg)

            # drain any remaining fillers (qg3 out-proj went out inline)
            for ent in fill_qkv + fill_out:
                ent[-1]()
            fill_qkv[:] = []
            fill_out[:] = []

    nc.compile()
    return nc


def _pack_w(w):
    # [DM, C] -> [128, MO, C]: partition p holds rows {mo*128 + p}
    return np.ascontiguousarray(
        w.reshape(MO, 128, w.shape[1]).transpose(1, 0, 2)
    ).astype(ml_dtypes.bfloat16)


def make_in_maps(x, w_qkv, b_qkv, w_out):
    # multiplicative post-exp mask: 1 where k <= q, else 0 (S^T layout)
    tri = np.tri(128, 128, 0, dtype=np.float32).T.astype(ml_dtypes.bfloat16)
    eye = np.eye(128, dtype=np.float32).astype(ml_dtypes.bfloat16)
    in_maps = []
    for core in range(8):
        b = core // 4
        hg = core % 4
        c0 = hg * CLOC
        csl = slice(c0, c0 + CLOC)

        cst = np.zeros((128, 264), np.uint16)
        cst[:, 0:128] = tri.view(np.uint16)
        cst[:, 128:256] = eye.view(np.uint16)
        bq = np.ascontiguousarray(
            b_qkv[csl].astype(np.float32).reshape(2, 128).T
        )
        bk = np.ascontiguousarray(
            b_qkv[DM + c0 : DM + c0 + CLOC].astype(np.float32).reshape(2, 128).T
        )
        cst[:, 256:260] = bq.view(np.uint16).reshape(128, 4)
        cst[:, 260:264] = bk.view(np.uint16).reshape(128, 4)

        wq_p = _pack_w(w_qkv[:, csl])
        wk_p = _pack_w(w_qkv[:, DM + c0 : DM + c0 + CLOC])
        # chunk-major: [128, 4, MO, 128], chunks (Qct0, Kct0, Qct1, Kct1)
        wqk = np.stack(
            [wq_p[:, :, 0:128], wk_p[:, :, 0:128],
             wq_p[:, :, 128:256], wk_p[:, :, 128:256]],
            axis=1,
        )
        in_maps.append(
            {
                "xT": _pack_w(np.ascontiguousarray(x[b].T)),
                "wqk": np.ascontiguousarray(wqk),
                "wv": _pack_w(w_qkv[:, 2 * DM + c0 : 2 * DM + c0 + CLOC]),
                "wo": np.ascontiguousarray(
                    w_out[csl, :].reshape(2, 128, DM).transpose(1, 0, 2)
                ).astype(ml_dtypes.bfloat16),
                "cst": cst.view(ml_dtypes.bfloat16),
            }
        )
    return in_maps


def gather(results, b_qkv, w_out, b_out):
    # device skips the V bias; z_norm + b_v projects to a constant row:
    # y += b_v @ w_out, folded into the output bias here
    b_eff = (
        b_out.astype(np.float32)
        + b_qkv[2 * DM :].astype(np.float32) @ w_out.astype(np.float32)
    )
    out = np.empty((B, S, DM), np.float32)
    for b in range(B):
        acc = results[4 * b]["y"].astype(np.float32)
        for j in range(1, 4):
            acc = acc + results[4 * b + j]["y"].astype(np.float32)
        out[b] = acc + b_eff[None, :]
    return out


def kernel(x, w_qkv, b_qkv, w_out, b_out):
    x = np.asarray(x)
    w_qkv = np.asarray(w_qkv)
    b_qkv = np.asarray(b_qkv)
    w_out = np.asarray(w_out)
    b_out = np.asarray(b_out)

    if "nc" not in _CACHE:
        _CACHE["nc"] = build()
    nc = _CACHE["nc"]

    in_maps = make_in_maps(x, w_qkv, b_qkv, w_out)
    res = run_bass_kernel_spmd(nc, in_maps, core_ids=list(range(8)))
    return gather(res.results, b_qkv, w_out, b_out)
